# revision 26
# baseline (speedup 1.0000x reference)
"""Causal multi-head attention on 8 Trainium2 cores (raw Bass).

Problem: x[4,2048,1024] @ W_qkv -> 16-head causal attention -> @ W_proj.
Sharding: core c handles batch b=c//2 and head-half c%2 (8 heads each).
Host pre-transposes x (feature-major xT) and pre-slices/scales weights;
each core computes its heads' contribution to out^T; host sums the two
half contributions per batch and adds b_proj.

Per-core pipeline (bf16/fp8 matmuls, fp32 PSUM), wavefront schedule:
  A1: qk^T[f,t] = w_qk^T @ x^T via fp8e4m3 DoubleRow 3-term compensated
      matmuls (hi*hi + xhat*wlo + xlo*what), per-block power-of-2 scales
      descaled in the DVE bias-add epilogue -> qk_sb bf16.
  Then four waves, one per 512-wide q/t chunk qc:
    B(all heads, qc): S^T = k^T.T @ q^T (bf16) on causal blocks,
      P^T = exp(S^T) on ACT -> bf16 (diag pair split to skip dead cols),
      triangle mask on diagonal 128-blocks (DVE), y_aug^T = V_aug^T @ P^T
      in PSUM (row 64 = sums), reciprocal (DVE) -> DMA sbuf broadcast
      [1,512]->[64,512] -> DVE multiply to normalize.  Odd heads staged
      per-chunk and DMA-shifted into ysb rows 64:128.
    A2(tg qc+1): V[t,f] = x @ w_v for the next wave's 4 t-tiles (fp8 DR
      3-term, psum banks 6/7) -- fills PE while B's exp tail drains.
    C(tc=qc): out^T = w_proj^T @ y^T (bf16) for this wave's q columns.
  Interleaving keeps the scalar engine's exp stream off the critical
  path: each wave has more PE work than ACT work.

build_nc(t, reps) can replicate the pipeline `reps` times in one NEFF
(serialized at rep boundaries) for wall-clock timing dilation.
"""

import contextlib
import math

import numpy as np
import ml_dtypes

import concourse.bass as bass
import concourse.mybir as mybir
from concourse.bass_utils import run_bass_kernel_spmd

F32 = mybir.dt.float32
BF16 = mybir.dt.bfloat16
F8 = mybir.dt.float8e4
ADD = mybir.AluOpType.add
MULT = mybir.AluOpType.mult
EXP = mybir.ActivationFunctionType.Exp
COPY = mybir.ActivationFunctionType.Copy
DRM = mybir.MatmulPerfMode.DoubleRow

NBF = ml_dtypes.bfloat16
NF8 = ml_dtypes.float8_e4m3

D_MODEL = 1024
D_K = 64
B, T = 4, 2048
NH = 8          # heads per core
KC = 8          # D_MODEL / 128
TQ = 512        # q-chunk width
N_CORES = 8

A_SC = 16.0     # x hi scale
BQ_SC = 256.0   # w scale, q features (pre-scaled by 1/sqrt(dk))
BK_SC = 32.0    # w scale, k features
BV_SC = 32.0    # w scale, v features
VARS = "hl"
# (w variant, x variant) term pairs: hi*hi + hi*lo + lo*hi
A_TERMS = [("h", "h"), ("l", "h"), ("h", "l")]


def build_nc(t=T, reps=1):
    tt_n = t // 128
    tc_n = t // TQ
    nc = bass.Bass(target_bir_lowering=False)

    x_d = {v: nc.dram_tensor(f"x{v}", [128, KC, t], F8,
                             kind="ExternalInput") for v in VARS}
    wqk_d = {v: nc.dram_tensor(f"wqk{v}", [128, KC, 8, 128], F8,
                               kind="ExternalInput") for v in VARS}
    wv_d = {v: nc.dram_tensor(f"wv{v}", [128, KC, 512], F8,
                              kind="ExternalInput") for v in VARS}
    wproj_d = {v: nc.dram_tensor(f"wproj{v}", [128, 4, 1024], F8,
                                 kind="ExternalInput") for v in VARS}
    bqk_d = nc.dram_tensor("bqk", [128, 8], F32, kind="ExternalInput")
    bv_d = nc.dram_tensor("bv", [128, 512], F32, kind="ExternalInput")
    tri_d = nc.dram_tensor("tri", [128, 128], BF16, kind="ExternalInput")
    out_d = nc.dram_tensor("outT", [128, 8, t], BF16, kind="ExternalOutput")

    # ---- schedule state ----
    prog = {"sync": [], "tensor": [], "vector": [], "scalar": []}
    cnt = {"pe": 0, "act": 0, "dve": 0}
    for _c in range(8):
        cnt[f"dma{_c}"] = 0
    last_wait = {e: {} for e in prog}
    bank_war = {}          # psum bank -> (sem, value): last consumer done
    FUSE = {"tensor", "vector", "scalar"}

    def op(engine, fn, waits=(), incs=()):
        w = []
        for s, v in waits:
            if v <= 0 or last_wait[engine].get(s, -1) >= v:
                continue
            last_wait[engine][s] = v
            w.append((s, v))
        prog[engine].append((fn, w, list(incs), engine in FUSE))
        for s, a in incs:
            cnt[s] += a

    NDMA = 8
    dma_rr = [0]

    def dma(dst, src, waits=()):
        ch = dma_rr[0] % NDMA
        dma_rr[0] += 1
        sem = f"dma{ch}"
        w = [(sem, cnt[sem])] + list(waits)   # chain within channel
        op("sync", lambda e, d=dst, s=src: e.dma_start(d, s),
           w, [(sem, 16)])
        return (sem, cnt[sem])

    stack = contextlib.ExitStack()
    sb = lambda name, shape, dt: stack.enter_context(
        nc.sbuf_tensor(name, shape, dt))

    # persistent region (x / wv stay resident for the wavefront A2)
    qk_sb = sb("qk", [128, 8, t], BF16)
    v_sb = sb("vsb", [128, tt_n, 8, 65], BF16)
    bqk_sb = sb("bqk_sb", [128, 8], F32)
    bv_sb = sb("bv_sb", [128, 512], F32)
    tri_sb = sb("tri_sb", [128, 128], BF16)
    x_sb = {v: sb(f"x{v}_sb", [128, KC, t], F8) for v in VARS}
    wv_sb = {v: sb(f"wv{v}_sb", [128, KC, 512], F8) for v in VARS}
    psum = stack.enter_context(nc.psum_tensor("ps", [128, 8, 512], F32))

    with contextlib.ExitStack() as semstack:
        semstack.enter_context(nc.allow_low_precision(
            reason="bf16/fp8 matmul operands and bf16 recip intentional"))
        sems = {}
        for _nm in ["pe", "act", "dve"] + [f"dma{_c}" for _c in range(8)]:
            sems[_nm] = semstack.enter_context(nc.semaphore(_nm + "_sem"))

        bqk_ret = dma(bqk_sb.ap(), bqk_d[:])
        bv_ret = dma(bv_sb.ap(), bv_d[:])
        tri_ret = dma(tri_sb.ap(), tri_d[:])

        # ones column of V_aug via DVE (x*0 + 1)
        op("vector",
           lambda e: e.tensor_scalar(
               v_sb.ap()[:, :, :, 64:65],
               bv_sb.ap()[:, 0:tt_n * 8].rearrange(
                   "p (a b c) -> p a b c", a=tt_n, b=8),
               0.0, 0.125, MULT, mybir.AluOpType.add),
           [bv_ret], [("dve", 1)])
        vones_ret = ("dve", cnt["dve"])

        # wqk stays resident (A1 is tc-wavefronted into the B waves)
        wqk_sb = {v: sb(f"wqk{v}_sb", [128, KC, 8, 128], F8)
                  for v in VARS}

        first_pv = [True]
        pair_war = {0: 0, 1: 0}
        slot_war = {0: 0, 1: 0}
        ybank_war = {}
        rsb_war = {}        # buf -> bcast dma ret (WAR for recip write)
        rep_war = {}        # buf -> dve cnt of norm (WAR for bcast write)
        ysbt_war = {}       # g -> shift dma ret (WAR for staging slot)
        pending_tail = []
        pending_pv = []
        pending_c = []
        b_alloc = [None]
        c_copy = {}
        c_dma = {}
        out_seq = [0]
        rep_gate = []
        dma_gate = []

        for rep in range(reps):
            # ---- phase A DMAs (kc-halves; T1's land first) ----
            xdma = {v: {} for v in VARS}
            wqdma = {v: {} for v in VARS}
            wvdma = {}
            for qt in range(4):
                ks = slice(2 * qt, 2 * qt + 2)
                xdma["h"][qt] = dma(
                    x_sb["h"].ap()[:, ks], x_d["h"][:, ks],
                    list(dma_gate))
                wqdma["h"][qt] = dma(
                    wqk_sb["h"].ap()[:, ks], wqk_d["h"][:, ks],
                    list(dma_gate))
            for hf in range(2):
                ks = slice(4 * hf, 4 * hf + 4)
                wqdma["l"][hf] = dma(
                    wqk_sb["l"].ap()[:, ks], wqk_d["l"][:, ks],
                    list(dma_gate))
                xdma["l"][hf] = dma(
                    x_sb["l"].ap()[:, ks], x_d["l"][:, ks],
                    list(dma_gate))
            for v_ in VARS:
                wvdma[v_] = dma(wv_sb[v_].ap(), wv_d[v_][:],
                                list(dma_gate))

            # ---- A1 helper: one (tc, grp, ftl) qk^T sub-tile ----
            a1_done = {}

            def emit_a1_sub(tc, grp, ftl, bank, defer_epi=False):
                ft = grp * 4 + ftl
                done = 0
                for ti, (wvar, xvar) in enumerate(A_TERMS):
                    for kp in range(4):
                        w = list(rep_gate)
                        xi = kp if xvar == "h" else kp // 2
                        wi = kp if wvar == "h" else kp // 2
                        w += [xdma[xvar][xi], wqdma[wvar][wi]]
                        if kp == 0 and ti == 0 and bank in bank_war:
                            w.append(bank_war.pop(bank))
                        last = (kp == 3 and ti == 2)
                        op("tensor",
                           lambda e, b=bank, f=ft, k=kp, tc_=tc,
                           wv_=wvar, xv_=xvar, st=(
                               kp == 0 and ti == 0), sp=last:
                               e.matmul(
                                   psum.ap()[:, b],
                                   wqk_sb[wv_].ap()[
                                       :, 2 * k:2 * k + 2, f],
                                   x_sb[xv_].ap()[
                                       :, 2 * k:2 * k + 2,
                                       tc_ * TQ:(tc_ + 1) * TQ],
                                   start=st, stop=sp,
                                   perf_mode=DRM),
                           w, [("pe", 1)] if last else [])
                        if last:
                            done = cnt["pe"]
                dsc = 1.0 / (A_SC * (BQ_SC if grp == 0 else BK_SC))

                def epi(bank=bank, ft=ft, tc=tc, dsc=dsc, grp=grp,
                        ftl=ftl, done=done):
                    op("vector",
                       lambda e, b=bank, f=ft, tc_=tc, c=dsc:
                           e.tensor_scalar(
                               qk_sb.ap()[:, f,
                                          tc_ * TQ:(tc_ + 1) * TQ],
                               psum.ap()[:, b],
                               c, bqk_sb.ap()[:, f:f + 1], MULT, ADD),
                       [("pe", done), bqk_ret], [("dve", 1)])
                    bank_war[bank] = ("dve", cnt["dve"])
                    a1_done[(tc, ft)] = ("dve", cnt["dve"])
                if defer_epi:
                    return epi
                epi()
                return None

            # pre-wave: only tc0, ladder order (head-pair g's q then k
            # block back-to-back) so wave0's heads start staggered
            for ftl in range(4):
                for grp in range(2):
                    emit_a1_sub(0, grp, ftl, grp * 4 + ftl)

            # ---- A2 helper: one V t-tile (psum banks 6/7) ----
            a2_done = {}

            def emit_a2_tile(tg, j, defer_epi=False, bank=None):
                tt = 4 * tg + j
                if bank is None:
                    bank = 6 + j % 2
                done = 0
                for ti, (wvar, xvar) in enumerate(A_TERMS):
                    for kp in range(4):
                        w = list(rep_gate)
                        w += [wvdma[wvar],
                              xdma[xvar][kp if xvar == "h" else kp // 2]]
                        if kp == 0 and ti == 0 and bank in bank_war:
                            w.append(bank_war.pop(bank))
                        last = (kp == 3 and ti == 2)
                        op("tensor",
                           lambda e, b=bank, k=kp, tt_=tt,
                           wv_=wvar, xv_=xvar,
                           st=(kp == 0 and ti == 0), sp=last:
                               e.matmul(
                                   psum.ap()[:, b],
                                   x_sb[xv_].ap()[
                                       :, 2 * k:2 * k + 2,
                                       tt_ * 128:(tt_ + 1) * 128],
                                   wv_sb[wv_].ap()[
                                       :, 2 * k:2 * k + 2],
                                   start=st, stop=sp,
                                   perf_mode=DRM),
                           w, [("pe", 1)] if last else [])
                        if last:
                            done = cnt["pe"]
                def epi(bank=bank, tt=tt, tg=tg, j=j, done=done):
                    op("vector",
                       lambda e, b=bank, tt_=tt:
                           e.scalar_tensor_tensor(
                               v_sb.ap()[:, tt_, :, 0:64],
                               psum.ap()[:, b], 1.0 / (A_SC * BV_SC),
                               bv_sb.ap()[:], MULT, ADD),
                       [("pe", done), bv_ret], [("dve", 1)])
                    bank_war[bank] = ("dve", cnt["dve"])
                    if j == 3:
                        a2_done[tg] = ("dve", cnt["dve"])
                if defer_epi:
                    return epi
                epi()
                return None

            if b_alloc[0] is None:
                ysb = {v: sb(f"ysb{v}", [128, 4, t], F8) for v in VARS}
                ysbt = {v: sb(f"ysbt{v}", [64, 4, 512], F8)
                        for v in VARS}
                pt_sb = sb("pt", [128, 4, 512], BF16)
                yun = sb("yun", [64, 2, 512], BF16)
                rsb = sb("rsb", [65, 2, 512], BF16)
                rep_sb = sb("repb", [64, 2, 512], BF16)
                t1_sb = sb("t1b", [64, 2, 512], BF16)
                osb = sb("osb", [128, 8, 512], BF16)
                wproj_sb = {v: sb(f"wproj{v}_sb", [128, 4, 1024], F8)
                            for v in VARS}
                b_alloc[0] = (ysb, ysbt, pt_sb, yun, rsb, rep_sb, t1_sb,
                              osb, wproj_sb)
            else:
                (ysb, ysbt, pt_sb, yun, rsb, rep_sb, t1_sb, osb,
                 wproj_sb) = b_alloc[0]

            # wproj may alias the wqk region: wait for A1's last read
            wproj_dma = [dma(wproj_sb[v].ap(), wproj_d[v][:],
                             list(rep_gate)) for v in VARS]

            for _j in range(4):
                emit_a2_tile(0, _j)

            # ---- phase C helpers (fp8 DR; one wave = 8 tiles) ----
            def c_mm(bank, gp, wvar, yvar, ft, tc_, st, sp):
                return lambda e, bk=bank, g=gp, wv_=wvar, yv_=yvar, \
                    f=ft, tc__=tc_, s=st, p=sp: \
                    e.matmul(
                        psum.ap()[:, bk],
                        wproj_sb[wv_].ap()[:, 2 * g:2 * g + 2,
                                           f * 128:(f + 1) * 128],
                        ysb[yv_].ap()[:, 2 * g:2 * g + 2,
                                      tc__ * TQ:(tc__ + 1) * TQ],
                        start=s, stop=p, perf_mode=DRM)

            def c_tile_start(j, bank, ft, tc_, ydone, act_fence):
                # act_fence: last exp of this wave reading psum banks 0..3
                w = list(wproj_dma) + [("act", act_fence)]
                w += ydone[0] + ydone[1]
                if j >= 4:
                    w.append(("act", c_copy[j - 4]))
                if bank in bank_war:
                    w.append(bank_war.pop(bank))
                for ti, (wvar, yvar) in enumerate(A_TERMS):
                    op("tensor",
                       c_mm(bank, 0, wvar, yvar, ft, tc_, ti == 0, False),
                       w if ti == 0 else [], [])

            def c_tile_end(j, bank, ft, tc_, ydone):
                w = ydone[2] + ydone[3]
                for ti, (wvar, yvar) in enumerate(A_TERMS):
                    op("tensor",
                       c_mm(bank, 1, wvar, yvar, ft, tc_, False, ti == 2),
                       w if ti == 0 else [],
                       [("pe", 1)] if ti == 2 else [])
                mm_done = cnt["pe"]
                w = [("pe", mm_done)]
                if j >= 8:
                    w.append(c_dma[j - 8])
                op("scalar",
                   lambda e, bk=bank, ob=j % 8: e.activation(
                       osb.ap()[:, ob], psum.ap()[:, bk], COPY,
                       scale=1.0 / 256.0),
                   w, [("act", 1)])
                c_copy[j] = cnt["act"]
                bank_war[bank] = ("act", cnt["act"])
                c_dma[j] = dma(
                    out_d[:, ft, tc_ * TQ:(tc_ + 1) * TQ],
                    osb.ap()[:, j % 8],
                    [("act", c_copy[j])])

            # ---- waves: B(all heads, qc) + A2(tg qc+1) + C(tc=qc) ----
            for qc in range(tc_n):
                ydone = {}        # g -> waits for this wave's C
                for hi in range(NH):
                    h = hi
                    g = h // 2
                    qrow = (h % 2) * 64
                    qf, kf = g, 4 + g
                    i = qc * NH + hi
                    yb = 4 + i % 2
                    nkt = 4 * qc + 4
                    npairs = 2 * qc + 2

                    def s_mm(kt, bank, qrow=qrow, kf=kf, qf=qf, qc=qc):
                        r = kt - 4 * qc
                        off = max(0, r * 128)
                        n = TQ - off
                        return lambda e, kt=kt, b=bank, off=off, n=n: \
                            e.matmul(
                                psum.ap()[:, b, off:off + n],
                                qk_sb.ap()[qrow:qrow + 64, kf,
                                           kt * 128:(kt + 1) * 128],
                                qk_sb.ap()[qrow:qrow + 64, qf,
                                           qc * TQ + off:qc * TQ + off + n],
                                start=True, stop=True)

                    def pv_mm(kt, slot, start, stop, h=h, qc=qc, yb=yb):
                        r = kt - 4 * qc
                        off = max(0, r * 128)
                        n = TQ - off
                        return lambda e, kt=kt, s=slot, off=off, n=n, \
                            st=start, sp=stop: e.matmul(
                                psum.ap()[0:65, yb, off:off + n],
                                v_sb.ap()[:, kt, h, :],
                                pt_sb.ap()[:, s, off:off + n],
                                start=st, stop=sp)

                    s_done = {}
                    pt_ready = {}
                    tails_old = []
                    fill_epis = []

                    for p in range(npairs):
                        pg = p % 2
                        kts = (2 * p, 2 * p + 1)
                        banks = (pg * 2, pg * 2 + 1)
                        w = [("act", pair_war[pg]),
                             a1_done[(qc, qf)], a1_done[(qc, kf)]]
                        for bq in banks:
                            if bq in bank_war:
                                w.append(bank_war.pop(bq))
                        op("tensor", s_mm(kts[0], banks[0]), w, [])
                        op("tensor", s_mm(kts[1], banks[1]), [],
                           [("pe", 1)])
                        s_done[p] = cnt["pe"]
                        if p == 0:
                            # flush the previous iteration's deferred final
                            # PV (+recip/bcast); norms wait until iteration
                            # end so their bcast-DMA waits don't block the
                            # masks on the in-order DVE queue
                            tails_old[:] = pending_tail
                            pending_tail.clear()
                            for _f in pending_pv:
                                _f()
                            pending_pv.clear()
                            # PE filler between this iteration's QK pairs
                            # and its first PV hides the exp-chain latency;
                            # epilogues deferred off the DVE queue
                            if qc + 1 < tc_n:
                                fill_epis.append(emit_a1_sub(
                                    qc + 1, hi % 2, hi // 2,
                                    6 + hi % 2, defer_epi=True))
                                if hi in (1, 3):
                                    # bank 6: interleaves with the even A1
                                    # sub-tiles; bank 7 would collide with
                                    # the still-open odd sub-tile group
                                    fill_epis.append(emit_a2_tile(
                                        qc + 1, (hi - 1) // 2,
                                        defer_epi=True, bank=6))
                        if p >= 1:
                            pp = p - 1
                            w = [pt_ready[pp], a2_done[qc]]
                            if first_pv[0]:
                                w += [vones_ret]
                                first_pv[0] = False
                            if pp == 0:
                                if yb in ybank_war:
                                    w.append(("dve", ybank_war[yb]))
                                if yb in bank_war:
                                    w.append(bank_war.pop(yb))
                            op("tensor",
                               pv_mm(2 * pp, (pp % 2) * 2,
                                     2 * pp == 0, False), w, [])
                            op("tensor",
                               pv_mm(2 * pp + 1, (pp % 2) * 2 + 1, False,
                                     2 * pp + 1 == nkt - 1),
                               [], [("pe", 1)])
                            slot_war[pp % 2] = cnt["pe"]
                        # exp; diag pair split to skip dead columns
                        diag = (kts[1] - 4 * qc) >= 0
                        r0 = kts[0] - 4 * qc
                        w = [("pe", s_done[p]), ("pe", slot_war[pg])]
                        if diag and r0 >= 2:
                            op("scalar",
                               lambda e, bq=banks[0], s=pg * 2, o=128 * r0:
                                   e.activation(
                                       pt_sb.ap()[:, s, o:TQ],
                                       psum.ap()[:, bq, o:TQ], EXP),
                               w, [])
                            op("scalar",
                               lambda e, bq=banks[1], s=pg * 2 + 1,
                               o=128 * (r0 + 1):
                                   e.activation(
                                       pt_sb.ap()[:, s, o:TQ],
                                       psum.ap()[:, bq, o:TQ], EXP),
                               [], [("act", 1)])
                        else:
                            off0 = max(0, r0) * 128
                            op("scalar",
                               lambda e, bq=banks[0], s=pg * 2, o=off0:
                                   e.activation(
                                       pt_sb.ap()[:, s:s + 2]
                                           .rearrange("p a b -> p (a b)")
                                           [:, o:2 * TQ],
                                       psum.ap()[:, bq:bq + 2]
                                           .rearrange("p a b -> p (a b)")
                                           [:, o:2 * TQ],
                                       EXP),
                               w, [("act", 1)])
                        pair_war[pg] = cnt["act"]
                        pt_ready[p] = ("act", cnt["act"])
                        if diag:
                            for j in (0, 1):
                                r = kts[j] - 4 * qc
                                if r < 0:
                                    continue
                                op("vector",
                                   lambda e, s=pg * 2 + j, r=r:
                                       e.tensor_tensor(
                                           pt_sb.ap()[:, s,
                                                      r * 128:r * 128 + 128],
                                           pt_sb.ap()[:, s,
                                                      r * 128:r * 128 + 128],
                                           tri_sb.ap()[:], MULT),
                                   [("act", pt_ready[p][1]), tri_ret],
                                   [("dve", 1)] if j == 1 else [])
                            pt_ready[p] = ("dve", cnt["dve"])

                    buf = i % 2
                    if h % 2 == 0:
                        out_h = ysb["h"].ap()[0:64, g,
                                              qc * TQ:(qc + 1) * TQ]
                        out_l = ysb["l"].ap()[0:64, g,
                                              qc * TQ:(qc + 1) * TQ]
                    else:
                        out_h = ysbt["h"].ap()[0:64, g, :]
                        out_l = ysbt["l"].ap()[0:64, g, :]

                    def _tail(out_h, out_l, buf, bret, h, g, qc, ydone):
                        def emit():
                            w = [bret]
                            if h % 2 == 1 and g in ysbt_war:
                                w += ysbt_war[g]
                            # t1 = 8*y; Yh = f8(t1); Yl = f8(t1 - Yh)
                            op("vector",
                               lambda e, b=buf:
                                   e.tensor_tensor(
                                       t1_sb.ap()[0:64, b],
                                       yun.ap()[0:64, b],
                                       rep_sb.ap()[0:64, b], MULT),
                               w, [("dve", 1)])
                            rep_war[buf] = cnt["dve"]
                            op("vector",
                               lambda e, o=out_h, b=buf: e.tensor_copy(
                                   o, t1_sb.ap()[0:64, b]),
                               [], [("dve", 1)])
                            op("vector",
                               lambda e, o=out_l, oh=out_h, b=buf:
                                   e.tensor_tensor(
                                       o, t1_sb.ap()[0:64, b], oh,
                                       mybir.AluOpType.subtract),
                               [], [("dve", 1)])
                            if h % 2 == 1:
                                nds = []
                                for v_ in VARS:
                                    nds.append(dma(
                                        ysb[v_].ap()[64:128, g,
                                                     qc * TQ:(qc + 1) * TQ],
                                        ysbt[v_].ap()[0:64, g, :],
                                        [("dve", cnt["dve"])]))
                                ysbt_war[g] = nds
                                ydone.setdefault(g, []).extend(nds)
                            else:
                                ydone.setdefault(g, []).append(
                                    ("dve", cnt["dve"]))
                        return emit

                    def _pv_final(pp=npairs - 1, ptr=pt_ready[npairs - 1],
                                  pv_mm=pv_mm, nkt=nkt, yb=yb, buf=buf,
                                  out_h=out_h, out_l=out_l, h=h, g=g,
                                  qc=qc, ydone=ydone):
                        def emit():
                            w = [ptr, a2_done[qc]]
                            if first_pv[0]:
                                w += [vones_ret]
                                first_pv[0] = False
                            op("tensor", pv_mm(2 * pp, (pp % 2) * 2,
                                               False, False), w, [])
                            op("tensor",
                               pv_mm(2 * pp + 1, (pp % 2) * 2 + 1,
                                     False, True), [], [("pe", 1)])
                            slot_war[pp % 2] = cnt["pe"]
                            pv_all = cnt["pe"]
                            # reciprocal of sums (row 64) -> bf16
                            w = [("pe", pv_all)]
                            if buf in rsb_war:
                                w.append(rsb_war[buf])
                            op("vector",
                               lambda e, yb=yb, b=buf: e.reciprocal(
                                   rsb.ap()[64:65, b],
                                   psum.ap()[64:65, yb]),
                               w, [("dve", 1)])
                            recip_done = cnt["dve"]
                            op("vector",
                               lambda e, yb=yb, b=buf: e.tensor_copy(
                                   yun.ap()[0:64, b], psum.ap()[0:64, yb]),
                               [], [("dve", 1)])
                            ybank_war[yb] = cnt["dve"]
                            # broadcast recip across 64 partitions via DMA
                            bw = [("dve", recip_done)]
                            if buf in rep_war:
                                bw.append(("dve", rep_war[buf]))
                            bsrc = rsb.ap()[64:65, buf] \
                                .rearrange("p (a n) -> p a n", a=1) \
                                .broadcast_to([1, 64, TQ])
                            bcast_ret = dma(rep_sb.ap()[0:64, buf],
                                            bsrc, bw)
                            rsb_war[buf] = bcast_ret
                            pending_tail.append(
                                _tail(out_h, out_l, buf, bcast_ret, h, g,
                                      qc, ydone))
                        return emit
                    pending_pv.append(_pv_final())
                    for _f in fill_epis:
                        _f()
                    for _t in tails_old:
                        _t()

                    # previous wave's C runs here: its tail drain has now
                    # completed under this wave's B work
                    if hi == 1 and pending_c:
                        for _f in pending_c:
                            _f()
                        pending_c.clear()

                # flush the last deferred PV first (its recip/bcast then
                # drain under the last A2 tile's PE block), then the norms
                tails_old = pending_tail[:]
                pending_tail.clear()
                for _f in pending_pv:
                    _f()
                pending_pv.clear()
                if qc + 1 < tc_n:
                    emit_a2_tile(qc + 1, 2)
                    emit_a2_tile(qc + 1, 3)
                for _t in tails_old:
                    _t()
                for _t in pending_tail:
                    _t()
                pending_tail.clear()

                # ---- phase C for this wave (deferred into next wave) ----
                yd = {g_: list(ydone[g_]) for g_ in range(4)}
                j0 = out_seq[0]
                out_seq[0] += 8

                def make_c(j0=j0, qc=qc, yd=yd):
                    def emit():
                        fence = cnt["act"]
                        defer = 3
                        pend = []
                        for ft in range(8):
                            j = j0 + ft
                            bank = j % 4
                            c_tile_start(j, bank, ft, qc, yd, fence)
                            if ft < defer:
                                pend.append((j, bank, ft, qc))
                                continue
                            while pend:
                                c_tile_end(*pend.pop(0), yd)
                            c_tile_end(j, bank, ft, qc, yd)
                        while pend:
                            c_tile_end(*pend.pop(0), yd)
                    return emit
                pending_c.append(make_c())
                if qc == tc_n - 1:
                    for _f in pending_c:
                        _f()
                    pending_c.clear()

            rep_gate = [("act", c_copy[out_seq[0] - 1]),
                        c_dma[out_seq[0] - 1]]
            dma_gate = [a1_done[(tc_n - 1, 7)], a2_done[tc_n - 1]]
            # seed psum WARs for next rep's A phase
            for bk in (4, 5):
                bank_war.setdefault(bk, ("dve", ybank_war.get(bk, 0)))

        # ---- emit ----
        with nc.Block() as block:
            def emitter(name):
                def run(eng):
                    for fn, waits, incs, fuse in prog[name]:
                        pre = waits[1:] if (fuse and waits) else waits
                        for s, v in pre:
                            eng.wait_ge(sems[s], v)
                        ins = fn(eng)
                        if fuse and waits:
                            s, v = waits[0]
                            ins.wait_op(sems[s], v, "sem-ge")
                        for s, a in incs:
                            ins.then_inc(sems[s], a)
                return run
            block.sync(emitter("sync"))
            block.tensor(emitter("tensor"))
            block.vector(emitter("vector"))
            block.scalar(emitter("scalar"))

    stack.close()
    return nc


# ---------------------------------------------------------------------------

def _f8(v):
    return np.ascontiguousarray(v).astype(NF8)


def _split8(v, hi_sc):
    """Return (hi, lo) fp8 arrays for scaled 3-term matmul."""
    hi = _f8(hi_sc * v)
    lo = _f8(hi_sc * v - hi.astype(np.float32))
    return hi, lo


def host_prep(x, W_qkv, b_qkv, W_proj, b_proj, t=T):
    scale = 1.0 / math.sqrt(D_K)
    x = np.asarray(x, np.float32)
    W_qkv = np.asarray(W_qkv, np.float32)
    b_qkv = np.asarray(b_qkv, np.float32)
    W_proj = np.asarray(W_proj, np.float32)

    tri = (np.arange(128)[None, :] >= np.arange(128)[:, None]) \
        .astype(NBF)

    in_maps = []
    for c in range(N_CORES):
        b = c // 2
        f0 = (c % 2) * 512
        xT = np.ascontiguousarray(
            x[b, :t].T.reshape(KC, 128, t).transpose(1, 0, 2))
        xh, xl = _split8(xT, A_SC)

        wq = W_qkv[:, f0:f0 + 512] * scale
        wk = W_qkv[:, D_MODEL + f0:D_MODEL + f0 + 512]

        def wlayout(w):  # [1024, 512] -> [128, KC, 4, 128]
            return np.ascontiguousarray(
                w.reshape(KC, 128, 4, 128).transpose(1, 0, 2, 3))

        wqk = {}
        qh, ql = _split8(wlayout(wq), BQ_SC)
        kh, kl = _split8(wlayout(wk), BK_SC)
        wqk["h"] = np.concatenate([qh, kh], axis=2)
        wqk["l"] = np.concatenate([ql, kl], axis=2)

        wv = W_qkv[:, 2 * D_MODEL + f0:2 * D_MODEL + f0 + 512]
        wv = np.ascontiguousarray(
            wv.reshape(KC, 128, 512).transpose(1, 0, 2))
        vh, vl = _split8(wv, BV_SC)

        bq = b_qkv[f0:f0 + 512] * scale
        bk_ = b_qkv[D_MODEL + f0:D_MODEL + f0 + 512]
        bqk = np.ascontiguousarray(
            np.concatenate([bq, bk_]).reshape(8, 128).T).astype(np.float32)
        bv = b_qkv[2 * D_MODEL + f0:2 * D_MODEL + f0 + 512]
        bv_rep = np.broadcast_to(bv, (128, 512)).astype(np.float32).copy()
        wp = W_proj[f0:f0 + 512]
        wp = np.ascontiguousarray(
            wp.reshape(4, 128, 1024).transpose(1, 0, 2))
        wph, wpl = _split8(wp, 32.0)
        in_maps.append({
            "xh": xh, "xl": xl,
            "wqkh": wqk["h"], "wqkl": wqk["l"],
            "wvh": vh, "wvl": vl,
            "wprojh": wph, "wprojl": wpl,
            "bqk": bqk, "bv": bv_rep, "tri": tri,
        })
    return in_maps


def host_gather(results, b_proj, t=T):
    b_proj = np.asarray(b_proj, np.float32)
    out = np.empty((B, t, D_MODEL), np.float32)
    for b in range(B):
        acc = None
        for half in range(2):
            r = results[2 * b + half]["outT"].astype(np.float32)
            oT = r.transpose(1, 0, 2).reshape(D_MODEL, t)
            acc = oT if acc is None else acc + oT
        out[b] = acc.T + b_proj
    return out


_NC_CACHE = {}


def kernel(x, W_qkv, b_qkv, W_proj, b_proj):
    if T not in _NC_CACHE:
        _NC_CACHE[T] = build_nc(T)
    nc = _NC_CACHE[T]
    in_maps = host_prep(x, W_qkv, b_qkv, W_proj, b_proj)
    res = run_bass_kernel_spmd(nc, in_maps, core_ids=list(range(N_CORES)))
    return host_gather(res.results, b_proj)


# revision 30
# speedup vs baseline: 1.4475x; 1.4475x over previous
"""Causal multi-head attention on 8 Trainium2 cores (raw Bass).

Problem: x[4,2048,1024] @ W_qkv -> 16-head causal attention -> @ W_proj.
Sharding: core c handles batch b=c//2 and head-half c%2 (8 heads each).
Host pre-transposes x (feature-major xT) and pre-slices/scales weights;
each core computes its heads' contribution to out^T; host sums the two
half contributions per batch and adds b_proj.

Per-core pipeline (bf16/fp8 matmuls, fp32 PSUM), wavefront schedule:
  A1: qk^T[f,t] = w_qk^T @ x^T via fp8e4m3 DoubleRow 3-term compensated
      matmuls (hi*hi + xhat*wlo + xlo*what), per-block power-of-2 scales
      descaled in the DVE bias-add epilogue -> qk_sb bf16.
  Then four waves, one per 512-wide q/t chunk qc:
    B(all heads, qc): S^T = k^T.T @ q^T (bf16) on causal blocks,
      P^T = exp(S^T) on ACT -> bf16 (diag pair split to skip dead cols),
      triangle mask on diagonal 128-blocks (DVE), y_aug^T = V_aug^T @ P^T
      in PSUM (row 64 = sums), reciprocal (DVE) -> DMA sbuf broadcast
      [1,512]->[64,512] -> DVE multiply to normalize.  Odd heads staged
      per-chunk and DMA-shifted into ysb rows 64:128.
    A2(tg qc+1): V[t,f] = x @ w_v for the next wave's 4 t-tiles (fp8 DR
      3-term, psum banks 6/7) -- fills PE while B's exp tail drains.
    C(tc=qc): out^T = w_proj^T @ y^T (bf16) for this wave's q columns.
  Interleaving keeps the scalar engine's exp stream off the critical
  path: each wave has more PE work than ACT work.

build_nc(t, reps) can replicate the pipeline `reps` times in one NEFF
(serialized at rep boundaries) for wall-clock timing dilation.
"""

import contextlib
import math

import numpy as np
import ml_dtypes

import concourse.bass as bass
import concourse.mybir as mybir
from concourse.bass_utils import run_bass_kernel_spmd

F32 = mybir.dt.float32
BF16 = mybir.dt.bfloat16
F8 = mybir.dt.float8e4
ADD = mybir.AluOpType.add
MULT = mybir.AluOpType.mult
EXP = mybir.ActivationFunctionType.Exp
COPY = mybir.ActivationFunctionType.Copy
DRM = mybir.MatmulPerfMode.DoubleRow

NBF = ml_dtypes.bfloat16
NF8 = ml_dtypes.float8_e4m3

D_MODEL = 1024
D_K = 64
B, T = 4, 2048
NH = 8          # heads per core
KC = 8          # D_MODEL / 128
TQ = 512        # q-chunk width
N_CORES = 8

A_SC = 16.0     # x hi scale
BQ_SC = 256.0   # w scale, q features (pre-scaled by 1/sqrt(dk))
BK_SC = 32.0    # w scale, k features
BV_SC = 32.0    # w scale, v features
VARS = "hl"
# (w variant, x variant) term pairs: hi*hi + hi*lo + lo*hi
A_TERMS = [("h", "h"), ("l", "h"), ("h", "l")]


def build_nc(t=T, reps=1):
    tt_n = t // 128
    tc_n = t // TQ
    nc = bass.Bass(target_bir_lowering=False)

    x_d = {v: nc.dram_tensor(f"x{v}", [128, KC, t], F8,
                             kind="ExternalInput") for v in VARS}
    wqk_d = {v: nc.dram_tensor(f"wqk{v}", [128, KC, 8, 128], F8,
                               kind="ExternalInput") for v in VARS}
    wv_d = {v: nc.dram_tensor(f"wv{v}", [128, KC, 512], F8,
                              kind="ExternalInput") for v in VARS}
    wproj_d = {v: nc.dram_tensor(f"wproj{v}", [128, 4, 1024], F8,
                                 kind="ExternalInput") for v in VARS}
    bqk_d = nc.dram_tensor("bqk", [128, 8], F32, kind="ExternalInput")
    bv_d = nc.dram_tensor("bv", [128, 512], F32, kind="ExternalInput")
    tri_d = nc.dram_tensor("tri", [128, 128], BF16, kind="ExternalInput")
    out_d = nc.dram_tensor("outT", [128, 8, t], BF16, kind="ExternalOutput")

    # ---- schedule state ----
    prog = {"sync": [], "tensor": [], "vector": [], "scalar": []}
    cnt = {"pe": 0, "act": 0, "dve": 0}
    for _c in range(8):
        cnt[f"dma{_c}"] = 0
    last_wait = {e: {} for e in prog}
    bank_war = {}          # psum bank -> (sem, value): last consumer done
    FUSE = {"tensor", "vector", "scalar"}

    def op(engine, fn, waits=(), incs=()):
        w = []
        for s, v in waits:
            if v <= 0 or last_wait[engine].get(s, -1) >= v:
                continue
            last_wait[engine][s] = v
            w.append((s, v))
        prog[engine].append((fn, w, list(incs), engine in FUSE))
        for s, a in incs:
            cnt[s] += a

    NDMA = 8
    dma_rr = [0]

    def dma(dst, src, waits=()):
        ch = dma_rr[0] % NDMA
        dma_rr[0] += 1
        sem = f"dma{ch}"
        w = [(sem, cnt[sem])] + list(waits)   # chain within channel
        op("sync", lambda e, d=dst, s=src: e.dma_start(d, s),
           w, [(sem, 16)])
        return (sem, cnt[sem])

    stack = contextlib.ExitStack()
    sb = lambda name, shape, dt: stack.enter_context(
        nc.sbuf_tensor(name, shape, dt))

    # persistent region (x / wv stay resident for the wavefront A2)
    qk_sb = sb("qk", [128, 8, t], BF16)
    v_sb = sb("vsb", [128, tt_n, 8, 65], BF16)
    bqk_sb = sb("bqk_sb", [128, 8], F32)
    bv_sb = sb("bv_sb", [128, 512], F32)
    tri_sb = sb("tri_sb", [128, 128], BF16)
    x_sb = {v: sb(f"x{v}_sb", [128, KC, t], F8) for v in VARS}
    wv_sb = {v: sb(f"wv{v}_sb", [128, KC, 512], F8) for v in VARS}
    psum = stack.enter_context(nc.psum_tensor("ps", [128, 8, 512], F32))

    with contextlib.ExitStack() as semstack:
        semstack.enter_context(nc.allow_low_precision(
            reason="bf16/fp8 matmul operands and bf16 recip intentional"))
        sems = {}
        for _nm in ["pe", "act", "dve"] + [f"dma{_c}" for _c in range(8)]:
            sems[_nm] = semstack.enter_context(nc.semaphore(_nm + "_sem"))

        bqk_ret = bv_ret = tri_ret = None
        vones_ret = None

        # wqk stays resident (A1 is tc-wavefronted into the B waves)
        wqk_sb = {v: sb(f"wqk{v}_sb", [128, KC, 8, 128], F8)
                  for v in VARS}

        first_pv = [True]
        pair_war = {0: 0, 1: 0}
        slot_war = {0: 0, 1: 0}
        ybank_war = {}
        rsb_war = {}        # buf -> bcast dma ret (WAR for recip write)
        rep_war = {}        # buf -> dve cnt of norm (WAR for bcast write)
        ysbt_war = {}       # g -> shift dma ret (WAR for staging slot)
        pending_tail = []
        pending_pv = []
        pending_c = []
        b_alloc = [None]
        c_copy = {}
        c_dma = {}
        out_seq = [0]
        rep_gate = []
        dma_gate = []

        for rep in range(reps):
            # ---- phase A DMAs (kc-halves; T1's land first) ----
            xdma = {v: {} for v in VARS}
            wqdma = {v: {} for v in VARS}
            wvdma = {}
            for qt in range(4):
                ks = slice(2 * qt, 2 * qt + 2)
                xdma["h"][qt] = dma(
                    x_sb["h"].ap()[:, ks], x_d["h"][:, ks],
                    list(dma_gate))
                wqdma["h"][qt] = dma(
                    wqk_sb["h"].ap()[:, ks], wqk_d["h"][:, ks],
                    list(dma_gate))
            for hf in range(2):
                ks = slice(4 * hf, 4 * hf + 4)
                wqdma["l"][hf] = dma(
                    wqk_sb["l"].ap()[:, ks], wqk_d["l"][:, ks],
                    list(dma_gate))
                xdma["l"][hf] = dma(
                    x_sb["l"].ap()[:, ks], x_d["l"][:, ks],
                    list(dma_gate))
            for v_ in VARS:
                wvdma[v_] = dma(wv_sb[v_].ap(), wv_d[v_][:],
                                list(dma_gate))
            if rep == 0:
                # constants issue after the critical A-phase inputs:
                # first consumers (epilogues/masks) run ~10us in
                bqk_ret = dma(bqk_sb.ap(), bqk_d[:])
                bv_ret = dma(bv_sb.ap(), bv_d[:])
                tri_ret = dma(tri_sb.ap(), tri_d[:])
                # ones column of V_aug via DVE (x*0 + 1/8)
                op("vector",
                   lambda e: e.tensor_scalar(
                       v_sb.ap()[:, :, :, 64:65],
                       bv_sb.ap()[:, 0:tt_n * 8].rearrange(
                           "p (a b c) -> p a b c", a=tt_n, b=8),
                       0.0, 0.125, MULT, mybir.AluOpType.add),
                   [bv_ret], [("dve", 1)])
                vones_ret = ("dve", cnt["dve"])

            # ---- A1 helper: one (tc, grp, ftl) qk^T sub-tile ----
            a1_done = {}

            def emit_a1_sub(tc, grp, ftl, bank, defer_epi=False):
                ft = grp * 4 + ftl
                done = 0
                for ti, (wvar, xvar) in enumerate(A_TERMS):
                    for kp in range(4):
                        w = list(rep_gate)
                        xi = kp if xvar == "h" else kp // 2
                        wi = kp if wvar == "h" else kp // 2
                        w += [xdma[xvar][xi], wqdma[wvar][wi]]
                        if kp == 0 and ti == 0 and bank in bank_war:
                            w.append(bank_war.pop(bank))
                        last = (kp == 3 and ti == 2)
                        op("tensor",
                           lambda e, b=bank, f=ft, k=kp, tc_=tc,
                           wv_=wvar, xv_=xvar, st=(
                               kp == 0 and ti == 0), sp=last:
                               e.matmul(
                                   psum.ap()[:, b],
                                   wqk_sb[wv_].ap()[
                                       :, 2 * k:2 * k + 2, f],
                                   x_sb[xv_].ap()[
                                       :, 2 * k:2 * k + 2,
                                       tc_ * TQ:(tc_ + 1) * TQ],
                                   start=st, stop=sp,
                                   perf_mode=DRM),
                           w, [("pe", 1)] if last else [])
                        if last:
                            done = cnt["pe"]
                dsc = 1.0 / (A_SC * (BQ_SC if grp == 0 else BK_SC))

                def epi(bank=bank, ft=ft, tc=tc, dsc=dsc, grp=grp,
                        ftl=ftl, done=done):
                    op("vector",
                       lambda e, b=bank, f=ft, tc_=tc, c=dsc:
                           e.tensor_scalar(
                               qk_sb.ap()[:, f,
                                          tc_ * TQ:(tc_ + 1) * TQ],
                               psum.ap()[:, b],
                               c, bqk_sb.ap()[:, f:f + 1], MULT, ADD),
                       [("pe", done), bqk_ret], [("dve", 1)])
                    bank_war[bank] = ("dve", cnt["dve"])
                    a1_done[(tc, ft)] = ("dve", cnt["dve"])
                if defer_epi:
                    return epi
                epi()
                return None

            # pre-wave: only tc0, ladder order (head-pair g's q then k
            # block back-to-back) so wave0's heads start staggered
            for ftl in range(4):
                for grp in range(2):
                    emit_a1_sub(0, grp, ftl, grp * 4 + ftl)

            # ---- A2 helper: one V t-tile (psum banks 6/7) ----
            a2_done = {}

            def emit_a2_tile(tg, j, defer_epi=False, bank=None):
                tt = 4 * tg + j
                if bank is None:
                    bank = 6 + j % 2
                done = 0
                for ti, (wvar, xvar) in enumerate(A_TERMS):
                    for kp in range(4):
                        w = list(rep_gate)
                        w += [wvdma[wvar],
                              xdma[xvar][kp if xvar == "h" else kp // 2]]
                        if kp == 0 and ti == 0 and bank in bank_war:
                            w.append(bank_war.pop(bank))
                        last = (kp == 3 and ti == 2)
                        op("tensor",
                           lambda e, b=bank, k=kp, tt_=tt,
                           wv_=wvar, xv_=xvar,
                           st=(kp == 0 and ti == 0), sp=last:
                               e.matmul(
                                   psum.ap()[:, b],
                                   x_sb[xv_].ap()[
                                       :, 2 * k:2 * k + 2,
                                       tt_ * 128:(tt_ + 1) * 128],
                                   wv_sb[wv_].ap()[
                                       :, 2 * k:2 * k + 2],
                                   start=st, stop=sp,
                                   perf_mode=DRM),
                           w, [("pe", 1)] if last else [])
                        if last:
                            done = cnt["pe"]
                def epi(bank=bank, tt=tt, tg=tg, j=j, done=done):
                    op("vector",
                       lambda e, b=bank, tt_=tt:
                           e.scalar_tensor_tensor(
                               v_sb.ap()[:, tt_, :, 0:64],
                               psum.ap()[:, b], 1.0 / (A_SC * BV_SC),
                               bv_sb.ap()[:], MULT, ADD),
                       [("pe", done), bv_ret], [("dve", 1)])
                    bank_war[bank] = ("dve", cnt["dve"])
                    if j == 3:
                        a2_done[tg] = ("dve", cnt["dve"])
                if defer_epi:
                    return epi
                epi()
                return None

            if b_alloc[0] is None:
                ysb = {v: sb(f"ysb{v}", [128, 4, t], F8) for v in VARS}
                ysbt = {v: sb(f"ysbt{v}", [64, 4, 512], F8)
                        for v in VARS}
                pt_sb = sb("pt", [128, 4, 512], BF16)
                yun = sb("yun", [64, 2, 512], BF16)
                rsb = sb("rsb", [65, 2, 512], BF16)
                rep_sb = sb("repb", [64, 2, 512], BF16)
                t1_sb = sb("t1b", [64, 2, 512], BF16)
                osb = sb("osb", [128, 8, 512], BF16)
                wproj_sb = {v: sb(f"wproj{v}_sb", [128, 4, 1024], F8)
                            for v in VARS}
                b_alloc[0] = (ysb, ysbt, pt_sb, yun, rsb, rep_sb, t1_sb,
                              osb, wproj_sb)
            else:
                (ysb, ysbt, pt_sb, yun, rsb, rep_sb, t1_sb, osb,
                 wproj_sb) = b_alloc[0]

            # wproj may alias the wqk region: wait for A1's last read
            wproj_dma = [dma(wproj_sb[v].ap(), wproj_d[v][:],
                             list(rep_gate)) for v in VARS]

            for _j in range(4):
                emit_a2_tile(0, _j)

            # ---- phase C helpers (fp8 DR; one wave = 8 tiles) ----
            def c_mm(bank, gp, wvar, yvar, ft, tc_, st, sp):
                return lambda e, bk=bank, g=gp, wv_=wvar, yv_=yvar, \
                    f=ft, tc__=tc_, s=st, p=sp: \
                    e.matmul(
                        psum.ap()[:, bk],
                        wproj_sb[wv_].ap()[:, 2 * g:2 * g + 2,
                                           f * 128:(f + 1) * 128],
                        ysb[yv_].ap()[:, 2 * g:2 * g + 2,
                                      tc__ * TQ:(tc__ + 1) * TQ],
                        start=s, stop=p, perf_mode=DRM)

            def c_tile_start(j, bank, ft, tc_, ydone, act_fence):
                # act_fence: last exp of this wave reading psum banks 0..3
                w = list(wproj_dma) + [("act", act_fence)]
                w += ydone[0] + ydone[1]
                if j >= 4:
                    w.append(("act", c_copy[j - 4]))
                if bank in bank_war:
                    w.append(bank_war.pop(bank))
                for ti, (wvar, yvar) in enumerate(A_TERMS):
                    op("tensor",
                       c_mm(bank, 0, wvar, yvar, ft, tc_, ti == 0, False),
                       w if ti == 0 else [], [])

            def c_tile_end(j, bank, ft, tc_, ydone):
                w = ydone[2] + ydone[3]
                for ti, (wvar, yvar) in enumerate(A_TERMS):
                    op("tensor",
                       c_mm(bank, 1, wvar, yvar, ft, tc_, False, ti == 2),
                       w if ti == 0 else [],
                       [("pe", 1)] if ti == 2 else [])
                mm_done = cnt["pe"]
                w = [("pe", mm_done)]
                if j >= 8:
                    w.append(c_dma[j - 8])
                op("scalar",
                   lambda e, bk=bank, ob=j % 8: e.activation(
                       osb.ap()[:, ob], psum.ap()[:, bk], COPY,
                       scale=1.0 / 256.0),
                   w, [("act", 1)])
                c_copy[j] = cnt["act"]
                bank_war[bank] = ("act", cnt["act"])
                c_dma[j] = dma(
                    out_d[:, ft, tc_ * TQ:(tc_ + 1) * TQ],
                    osb.ap()[:, j % 8],
                    [("act", c_copy[j])])

            # ---- waves: B(all heads, qc) + A2(tg qc+1) + C(tc=qc) ----
            for qc in range(tc_n):
                ydone = {}        # g -> waits for this wave's C
                for hi in range(NH):
                    h = hi
                    g = h // 2
                    qrow = (h % 2) * 64
                    qf, kf = g, 4 + g
                    i = qc * NH + hi
                    yb = 4 + i % 2
                    nkt = 4 * qc + 4
                    npairs = 2 * qc + 2

                    def s_mm(kt, bank, qrow=qrow, kf=kf, qf=qf, qc=qc):
                        r = kt - 4 * qc
                        off = max(0, r * 128)
                        n = TQ - off
                        return lambda e, kt=kt, b=bank, off=off, n=n: \
                            e.matmul(
                                psum.ap()[:, b, off:off + n],
                                qk_sb.ap()[qrow:qrow + 64, kf,
                                           kt * 128:(kt + 1) * 128],
                                qk_sb.ap()[qrow:qrow + 64, qf,
                                           qc * TQ + off:qc * TQ + off + n],
                                start=True, stop=True)

                    def pv_mm(kt, slot, start, stop, h=h, qc=qc, yb=yb):
                        r = kt - 4 * qc
                        off = max(0, r * 128)
                        n = TQ - off
                        return lambda e, kt=kt, s=slot, off=off, n=n, \
                            st=start, sp=stop: e.matmul(
                                psum.ap()[0:65, yb, off:off + n],
                                v_sb.ap()[:, kt, h, :],
                                pt_sb.ap()[:, s, off:off + n],
                                start=st, stop=sp)

                    s_done = {}
                    pt_ready = {}
                    tails_old = []
                    fill_epis = []

                    for p in range(npairs):
                        pg = p % 2
                        kts = (2 * p, 2 * p + 1)
                        banks = (pg * 2, pg * 2 + 1)
                        w = [("act", pair_war[pg]),
                             a1_done[(qc, qf)], a1_done[(qc, kf)]]
                        for bq in banks:
                            if bq in bank_war:
                                w.append(bank_war.pop(bq))
                        op("tensor", s_mm(kts[0], banks[0]), w, [])
                        op("tensor", s_mm(kts[1], banks[1]), [],
                           [("pe", 1)])
                        s_done[p] = cnt["pe"]
                        if p == 0:
                            # flush the previous iteration's deferred final
                            # PV (+recip/bcast); norms wait until iteration
                            # end so their bcast-DMA waits don't block the
                            # masks on the in-order DVE queue
                            tails_old[:] = pending_tail
                            pending_tail.clear()
                            for _f in pending_pv:
                                _f()
                            pending_pv.clear()
                            # PE filler between this iteration's QK pairs
                            # and its first PV hides the exp-chain latency;
                            # epilogues deferred off the DVE queue
                            if qc + 1 < tc_n:
                                fill_epis.append(emit_a1_sub(
                                    qc + 1, hi % 2, hi // 2,
                                    6 + hi % 2, defer_epi=True))
                                if hi in (1, 3):
                                    # bank 6: interleaves with the even A1
                                    # sub-tiles; bank 7 would collide with
                                    # the still-open odd sub-tile group
                                    fill_epis.append(emit_a2_tile(
                                        qc + 1, (hi - 1) // 2,
                                        defer_epi=True, bank=6))
                        if p >= 1:
                            pp = p - 1
                            w = [pt_ready[pp], a2_done[qc]]
                            if first_pv[0]:
                                w += [vones_ret]
                                first_pv[0] = False
                            if pp == 0:
                                if yb in ybank_war:
                                    w.append(("dve", ybank_war[yb]))
                                if yb in bank_war:
                                    w.append(bank_war.pop(yb))
                            op("tensor",
                               pv_mm(2 * pp, (pp % 2) * 2,
                                     2 * pp == 0, False), w, [])
                            op("tensor",
                               pv_mm(2 * pp + 1, (pp % 2) * 2 + 1, False,
                                     2 * pp + 1 == nkt - 1),
                               [], [("pe", 1)])
                            slot_war[pp % 2] = cnt["pe"]
                        # exp; diag pair split to skip dead columns
                        diag = (kts[1] - 4 * qc) >= 0
                        r0 = kts[0] - 4 * qc
                        w = [("pe", s_done[p]), ("pe", slot_war[pg])]
                        if diag and r0 >= 2:
                            op("scalar",
                               lambda e, bq=banks[0], s=pg * 2, o=128 * r0:
                                   e.activation(
                                       pt_sb.ap()[:, s, o:TQ],
                                       psum.ap()[:, bq, o:TQ], EXP),
                               w, [])
                            op("scalar",
                               lambda e, bq=banks[1], s=pg * 2 + 1,
                               o=128 * (r0 + 1):
                                   e.activation(
                                       pt_sb.ap()[:, s, o:TQ],
                                       psum.ap()[:, bq, o:TQ], EXP),
                               [], [("act", 1)])
                        else:
                            off0 = max(0, r0) * 128
                            op("scalar",
                               lambda e, bq=banks[0], s=pg * 2, o=off0:
                                   e.activation(
                                       pt_sb.ap()[:, s:s + 2]
                                           .rearrange("p a b -> p (a b)")
                                           [:, o:2 * TQ],
                                       psum.ap()[:, bq:bq + 2]
                                           .rearrange("p a b -> p (a b)")
                                           [:, o:2 * TQ],
                                       EXP),
                               w, [("act", 1)])
                        pair_war[pg] = cnt["act"]
                        pt_ready[p] = ("act", cnt["act"])
                        if diag:
                            for j in (0, 1):
                                r = kts[j] - 4 * qc
                                if r < 0:
                                    continue
                                op("vector",
                                   lambda e, s=pg * 2 + j, r=r:
                                       e.tensor_tensor(
                                           pt_sb.ap()[:, s,
                                                      r * 128:r * 128 + 128],
                                           pt_sb.ap()[:, s,
                                                      r * 128:r * 128 + 128],
                                           tri_sb.ap()[:], MULT),
                                   [("act", pt_ready[p][1]), tri_ret],
                                   [("dve", 1)] if j == 1 else [])
                            pt_ready[p] = ("dve", cnt["dve"])

                    buf = i % 2
                    if h % 2 == 0:
                        out_h = ysb["h"].ap()[0:64, g,
                                              qc * TQ:(qc + 1) * TQ]
                        out_l = ysb["l"].ap()[0:64, g,
                                              qc * TQ:(qc + 1) * TQ]
                    else:
                        out_h = ysbt["h"].ap()[0:64, g, :]
                        out_l = ysbt["l"].ap()[0:64, g, :]

                    def _tail(out_h, out_l, buf, bret, h, g, qc, ydone):
                        def emit():
                            w = [bret]
                            if h % 2 == 1 and g in ysbt_war:
                                w += ysbt_war[g]
                            # t1 = 8*y; Yh = f8(t1); Yl = f8(t1 - Yh)
                            op("vector",
                               lambda e, b=buf:
                                   e.tensor_tensor(
                                       t1_sb.ap()[0:64, b],
                                       yun.ap()[0:64, b],
                                       rep_sb.ap()[0:64, b], MULT),
                               w, [("dve", 1)])
                            rep_war[buf] = cnt["dve"]
                            op("vector",
                               lambda e, o=out_h, b=buf: e.tensor_copy(
                                   o, t1_sb.ap()[0:64, b]),
                               [], [("dve", 1)])
                            op("vector",
                               lambda e, o=out_l, oh=out_h, b=buf:
                                   e.tensor_tensor(
                                       o, t1_sb.ap()[0:64, b], oh,
                                       mybir.AluOpType.subtract),
                               [], [("dve", 1)])
                            if h % 2 == 1:
                                nds = []
                                for v_ in VARS:
                                    nds.append(dma(
                                        ysb[v_].ap()[64:128, g,
                                                     qc * TQ:(qc + 1) * TQ],
                                        ysbt[v_].ap()[0:64, g, :],
                                        [("dve", cnt["dve"])]))
                                ysbt_war[g] = nds
                                ydone.setdefault(g, []).extend(nds)
                            else:
                                ydone.setdefault(g, []).append(
                                    ("dve", cnt["dve"]))
                        return emit

                    def _pv_final(pp=npairs - 1, ptr=pt_ready[npairs - 1],
                                  pv_mm=pv_mm, nkt=nkt, yb=yb, buf=buf,
                                  out_h=out_h, out_l=out_l, h=h, g=g,
                                  qc=qc, ydone=ydone):
                        def emit():
                            w = [ptr, a2_done[qc]]
                            if first_pv[0]:
                                w += [vones_ret]
                                first_pv[0] = False
                            op("tensor", pv_mm(2 * pp, (pp % 2) * 2,
                                               False, False), w, [])
                            op("tensor",
                               pv_mm(2 * pp + 1, (pp % 2) * 2 + 1,
                                     False, True), [], [("pe", 1)])
                            slot_war[pp % 2] = cnt["pe"]
                            pv_all = cnt["pe"]
                            # reciprocal of sums (row 64) -> bf16
                            w = [("pe", pv_all)]
                            if buf in rsb_war:
                                w.append(rsb_war[buf])
                            op("vector",
                               lambda e, yb=yb, b=buf: e.reciprocal(
                                   rsb.ap()[64:65, b],
                                   psum.ap()[64:65, yb]),
                               w, [("dve", 1)])
                            recip_done = cnt["dve"]
                            op("vector",
                               lambda e, yb=yb, b=buf: e.tensor_copy(
                                   yun.ap()[0:64, b], psum.ap()[0:64, yb]),
                               [], [("dve", 1)])
                            ybank_war[yb] = cnt["dve"]
                            # broadcast recip across 64 partitions via DMA
                            bw = [("dve", recip_done)]
                            if buf in rep_war:
                                bw.append(("dve", rep_war[buf]))
                            bsrc = rsb.ap()[64:65, buf] \
                                .rearrange("p (a n) -> p a n", a=1) \
                                .broadcast_to([1, 64, TQ])
                            bcast_ret = dma(rep_sb.ap()[0:64, buf],
                                            bsrc, bw)
                            rsb_war[buf] = bcast_ret
                            pending_tail.append(
                                _tail(out_h, out_l, buf, bcast_ret, h, g,
                                      qc, ydone))
                        return emit
                    pending_pv.append(_pv_final())
                    for _f in fill_epis:
                        _f()
                    for _t in tails_old:
                        _t()

                    # previous wave's C runs here: its tail drain has now
                    # completed under this wave's B work
                    if hi == 1 and pending_c:
                        for _f in pending_c:
                            _f()
                        pending_c.clear()

                # flush the last deferred PV first (its recip/bcast then
                # drain under the last A2 tile's PE block), then the norms
                tails_old = pending_tail[:]
                pending_tail.clear()
                for _f in pending_pv:
                    _f()
                pending_pv.clear()
                if qc + 1 < tc_n:
                    emit_a2_tile(qc + 1, 2)
                    emit_a2_tile(qc + 1, 3)
                for _t in tails_old:
                    _t()
                for _t in pending_tail:
                    _t()
                pending_tail.clear()

                # ---- phase C for this wave (deferred into next wave) ----
                yd = {g_: list(ydone[g_]) for g_ in range(4)}
                j0 = out_seq[0]
                out_seq[0] += 8

                def make_c(j0=j0, qc=qc, yd=yd):
                    def emit():
                        fence = cnt["act"]
                        defer = 3
                        pend = []
                        for ft in range(8):
                            j = j0 + ft
                            bank = j % 4
                            c_tile_start(j, bank, ft, qc, yd, fence)
                            if ft < defer:
                                pend.append((j, bank, ft, qc))
                                continue
                            while pend:
                                c_tile_end(*pend.pop(0), yd)
                            c_tile_end(j, bank, ft, qc, yd)
                        while pend:
                            c_tile_end(*pend.pop(0), yd)
                    return emit
                pending_c.append(make_c())
                if qc == tc_n - 1:
                    for _f in pending_c:
                        _f()
                    pending_c.clear()

            rep_gate = [("act", c_copy[out_seq[0] - 1]),
                        c_dma[out_seq[0] - 1]]
            dma_gate = [a1_done[(tc_n - 1, 7)], a2_done[tc_n - 1]]
            # seed psum WARs for next rep's A phase
            for bk in (4, 5):
                bank_war.setdefault(bk, ("dve", ybank_war.get(bk, 0)))

        # ---- emit ----
        with nc.Block() as block:
            def emitter(name):
                def run(eng):
                    for fn, waits, incs, fuse in prog[name]:
                        pre = waits[1:] if (fuse and waits) else waits
                        for s, v in pre:
                            eng.wait_ge(sems[s], v)
                        ins = fn(eng)
                        if fuse and waits:
                            s, v = waits[0]
                            ins.wait_op(sems[s], v, "sem-ge")
                        for s, a in incs:
                            ins.then_inc(sems[s], a)
                return run
            block.sync(emitter("sync"))
            block.tensor(emitter("tensor"))
            block.vector(emitter("vector"))
            block.scalar(emitter("scalar"))

    stack.close()
    return nc


# ---------------------------------------------------------------------------

def _f8(v):
    return np.ascontiguousarray(v).astype(NF8)


def _split8(v, hi_sc):
    """Return (hi, lo) fp8 arrays for scaled 3-term matmul."""
    hi = _f8(hi_sc * v)
    lo = _f8(hi_sc * v - hi.astype(np.float32))
    return hi, lo


def host_prep(x, W_qkv, b_qkv, W_proj, b_proj, t=T):
    scale = 1.0 / math.sqrt(D_K)
    x = np.asarray(x, np.float32)
    W_qkv = np.asarray(W_qkv, np.float32)
    b_qkv = np.asarray(b_qkv, np.float32)
    W_proj = np.asarray(W_proj, np.float32)

    tri = (np.arange(128)[None, :] >= np.arange(128)[:, None]) \
        .astype(NBF)

    in_maps = []
    for c in range(N_CORES):
        b = c // 2
        f0 = (c % 2) * 512
        xT = np.ascontiguousarray(
            x[b, :t].T.reshape(KC, 128, t).transpose(1, 0, 2))
        xh, xl = _split8(xT, A_SC)

        wq = W_qkv[:, f0:f0 + 512] * scale
        wk = W_qkv[:, D_MODEL + f0:D_MODEL + f0 + 512]

        def wlayout(w):  # [1024, 512] -> [128, KC, 4, 128]
            return np.ascontiguousarray(
                w.reshape(KC, 128, 4, 128).transpose(1, 0, 2, 3))

        wqk = {}
        qh, ql = _split8(wlayout(wq), BQ_SC)
        kh, kl = _split8(wlayout(wk), BK_SC)
        wqk["h"] = np.concatenate([qh, kh], axis=2)
        wqk["l"] = np.concatenate([ql, kl], axis=2)

        wv = W_qkv[:, 2 * D_MODEL + f0:2 * D_MODEL + f0 + 512]
        wv = np.ascontiguousarray(
            wv.reshape(KC, 128, 512).transpose(1, 0, 2))
        vh, vl = _split8(wv, BV_SC)

        bq = b_qkv[f0:f0 + 512] * scale
        bk_ = b_qkv[D_MODEL + f0:D_MODEL + f0 + 512]
        bqk = np.ascontiguousarray(
            np.concatenate([bq, bk_]).reshape(8, 128).T).astype(np.float32)
        bv = b_qkv[2 * D_MODEL + f0:2 * D_MODEL + f0 + 512]
        bv_rep = np.broadcast_to(bv, (128, 512)).astype(np.float32).copy()
        wp = W_proj[f0:f0 + 512]
        wp = np.ascontiguousarray(
            wp.reshape(4, 128, 1024).transpose(1, 0, 2))
        wph, wpl = _split8(wp, 32.0)
        in_maps.append({
            "xh": xh, "xl": xl,
            "wqkh": wqk["h"], "wqkl": wqk["l"],
            "wvh": vh, "wvl": vl,
            "wprojh": wph, "wprojl": wpl,
            "bqk": bqk, "bv": bv_rep, "tri": tri,
        })
    return in_maps


def host_gather(results, b_proj, t=T):
    b_proj = np.asarray(b_proj, np.float32)
    out = np.empty((B, t, D_MODEL), np.float32)
    for b in range(B):
        acc = None
        for half in range(2):
            r = results[2 * b + half]["outT"].astype(np.float32)
            oT = r.transpose(1, 0, 2).reshape(D_MODEL, t)
            acc = oT if acc is None else acc + oT
        out[b] = acc.T + b_proj
    return out


_NC_CACHE = {}


def kernel(x, W_qkv, b_qkv, W_proj, b_proj):
    if T not in _NC_CACHE:
        _NC_CACHE[T] = build_nc(T)
    nc = _NC_CACHE[T]
    in_maps = host_prep(x, W_qkv, b_qkv, W_proj, b_proj)
    res = run_bass_kernel_spmd(nc, in_maps, core_ids=list(range(N_CORES)))
    return host_gather(res.results, b_proj)
